# revision 1
# baseline (speedup 1.0000x reference)
"""Trainium2 Bass kernel for the aperiodic real-space Ewald sum (N=4096).

Math: with w_ij = erf(d_ij/sqrt(2)) / (d_ij + eps) (symmetric),
    t_j   = sum_i q_i w_ij
    field = t/(2*pi) + 2*SELF_C*q
    pot   = (q . t)/(4*pi) + SELF_C*sum(q^2)
Each core computes column sums of its 512-row block of z_ij = q_i * w_ij;
the host assembles field/pot from the gathered t. No device collectives.

Per-core device program (SPMD; every core's j columns are rotated host-side
so its own diagonal block sits at a fixed local offset, keeping one program
for all cores). This runtime charges ~40-50us for every instruction that
carries a semaphore wait while waitless same-engine ops are nearly free, so
the program is 4 mega-units of [128,4096] PSUM (all 8 banks) with one wait
per engine per unit:
  PE : d^2 = |p_i|^2+|p_j|^2-2 p_i.p_j as 8 fp32 K=5 matmuls (exact), plus a
       bf16 BIG*I matmul accumulated onto the diagonal block so the self-pair
       contributes ~2^-20 * q_i instead of sqrt(cancellation-noise).
  ACT: d = Sqrt(d^2); rq = AbsRsqrt(d^2 * (1/q_i^2)) = |q_i|/d;
       conv = Erf(d * (c*sign(q_i))) = sign(q_i)*erf(c*d)   (erf is odd)
  GP : z = conv*rq = q_i*erf(c d)/d; partition-reduce -> row of tpart; DMA.
"""
import sys

sys.path.insert(0, "/opt/trn_rl_repo")

import numpy as np
import ml_dtypes

import concourse.bass as bass
import concourse.tile as tile
from concourse import bacc, mybir
from concourse.bass_utils import run_bass_kernel_spmd

N = 4096
NCORES = 8
R = N // NCORES          # rows per core
CH = R // 128            # 128-row chunks per core
SIGMA = 1.0
TWOPI = 2.0 * np.pi
SELF_C = 1.0 / (SIGMA * TWOPI**1.5)
INV_SQRT2 = float(1.0 / np.sqrt(2.0))
BIG = float(2.0**40)

_nc_cache = None


def _build_nc(loop_n=None, unroll=1):
    nc = bacc.Bacc("TRN2", target_bir_lowering=False, debug=False,
                   num_devices=NCORES)
    f32 = mybir.dt.float32
    bf16 = mybir.dt.bfloat16
    E = mybir.ActivationFunctionType
    mult = mybir.AluOpType.mult

    aug_d = nc.dram_tensor("aug", [5, R + N], f32, kind="ExternalInput").ap()
    scl_d = nc.dram_tensor("scl", [128, 2 * CH], f32, kind="ExternalInput").ap()
    idn_d = nc.dram_tensor("idn", [128, 256], bf16, kind="ExternalInput").ap()
    tpart_d = nc.dram_tensor("tpart", [CH, N], f32, kind="ExternalOutput").ap()

    with tile.TileContext(nc) as tc:
        with (
            tc.tile_pool(name="sbin", bufs=1) as sbin,
            tc.tile_pool(name="psum", bufs=1, space="PSUM") as psum,
            tc.tile_pool(name="sbd", bufs=1) as sbd,
            tc.tile_pool(name="sbr", bufs=2) as sbr,
            tc.tile_pool(name="sbc", bufs=2) as sbc,
            tc.tile_pool(name="sbz", bufs=1) as sbz,
        ):
            aug = sbin.tile([5, R + N], f32, tag="aug")
            nc.sync.dma_start(aug[:], aug_d[:])
            scl = sbin.tile([128, 2 * CH], f32, tag="scl")
            nc.sync.dma_start(scl[:], scl_d[:])
            idn = sbin.tile([128, 256], bf16, tag="idn")
            nc.sync.dma_start(idn[:], idn_d[:])

            def body():
                for ic in range(CH):
                    p = psum.tile([128, N], f32, tag="p")
                    for k4 in range(N // 512):
                        nc.tensor.matmul(
                            p[:, k4 * 512:(k4 + 1) * 512],
                            aug[:, ic * 128:(ic + 1) * 128],
                            aug[:, R + k4 * 512:R + (k4 + 1) * 512],
                            start=True, stop=not (k4 == 0))
                        if k4 == 0:
                            nc.tensor.matmul(
                                p[:, ic * 128:(ic + 1) * 128],
                                idn[:, 0:128], idn[:, 128:256],
                                start=False, stop=True)
                    d = sbd.tile([128, N], f32, tag="d")
                    nc.scalar.activation(d[:], p[:], E.Sqrt)
                    rq = sbr.tile([128, N], f32, tag="rq")
                    nc.scalar.activation(rq[:], p[:], E.Abs_reciprocal_sqrt,
                                         scale=scl[:, CH + ic:CH + ic + 1])
                    conv = sbc.tile([128, N], f32, tag="conv")
                    nc.scalar.activation(conv[:], d[:], E.Erf,
                                         scale=scl[:, ic:ic + 1])
                    z = sbz.tile([128, N], f32, tag="z")
                    nc.vector.tensor_tensor(z[:], conv[:], rq[:], op=mult)
                    red = sbz.tile([1, N], f32, tag="red")
                    nc.gpsimd.tensor_reduce(red[:], z[:],
                                            axis=mybir.AxisListType.C,
                                            op=mybir.AluOpType.add)
                    nc.sync.dma_start(tpart_d[ic:ic + 1, :], red[0:1, :])

            if loop_n is not None:
                with tc.For_i(0, loop_n, 1):
                    for _ in range(unroll):
                        body()
            else:
                body()
    nc.compile()
    return nc


def _prep_inputs(positions, q):
    pos = np.ascontiguousarray(np.asarray(positions, dtype=np.float32))
    qv = np.asarray(q, dtype=np.float32).reshape(-1)
    s = (pos * pos).sum(axis=1, dtype=np.float32)

    rhs_all = np.empty((5, N), np.float32)
    rhs_all[0:3] = pos.T
    rhs_all[3] = 1.0
    rhs_all[4] = s

    idn = np.concatenate([np.eye(128), np.eye(128) * BIG],
                         axis=1).astype(ml_dtypes.bfloat16)

    in_maps = []
    for c in range(NCORES):
        blk = slice(c * R, (c + 1) * R)
        aug = np.empty((5, R + N), np.float32)
        aug[0:3, 0:R] = -2.0 * pos[blk].T
        aug[3, 0:R] = s[blk]
        aug[4, 0:R] = 1.0
        aug[:, R:] = np.roll(rhs_all, -c * R, axis=1)
        qb = qv[blk].reshape(CH, 128).T
        scl = np.empty((128, 2 * CH), np.float32)
        scl[:, 0:CH] = INV_SQRT2 * np.sign(qb)
        scl[:, CH:] = (1.0 / (qb.astype(np.float64) ** 2)).astype(np.float32)
        in_maps.append({"aug": aug, "scl": scl, "idn": idn})
    return in_maps, qv


def kernel(positions, q):
    global _nc_cache
    if _nc_cache is None:
        _nc_cache = _build_nc()
    nc = _nc_cache

    in_maps, qv = _prep_inputs(positions, q)
    res = run_bass_kernel_spmd(nc, in_maps, core_ids=list(range(NCORES)))

    t = np.zeros(N, np.float64)
    for c in range(NCORES):
        tp = res.results[c]["tpart"].astype(np.float64)  # local j order
        t += np.roll(tp.sum(axis=0), c * R)

    q64 = qv.astype(np.float64)
    field = t / TWOPI + 2.0 * SELF_C * q64
    pot = float((q64 * t).sum() / (2.0 * TWOPI) + SELF_C * (q64 * q64).sum())
    out = np.empty(N + 1, np.float32)
    out[0] = pot
    out[1:] = field.astype(np.float32)
    return out



# revision 3
# speedup vs baseline: 10.7147x; 10.7147x over previous
"""Trainium2 Bass kernel for the aperiodic real-space Ewald sum (N=4096).

Math: with w_ij = erf(d_ij/sqrt(2)) / (d_ij + eps) (symmetric),
    t_i   = sum_j q_j w_ij
    field = t/(2*pi) + 2*SELF_C*q
    pot   = (q . t)/(4*pi) + SELF_C*sum(q^2)

Sharding: core c owns rows [c*512, (c+1)*512). Each 128-row chunk computes
its [128, 4096] block of w against ALL columns j, multiplies by q_j along
the free axis, and reduces along the free axis (DVE accum_out) -- giving
t for its own rows directly. No partition reduction (gpsimd) and no
cross-core combining are needed; the host just concatenates row segments.

Per-core device program per chunk (4 chunks of 128 rows):
  PE : d^2 = |p_i|^2+|p_j|^2-2 p_i.p_j as 8 fp32 K=5 matmuls into one
       [128,4096] PSUM tile (all 8 banks), plus a bf16 BIG*I matmul on the
       diagonal block so the self-pair lands at d=2^20 (w_ii ~ 2^-20 ~ 0).
  ACT: d = Sqrt(d^2); r = AbsRsqrt(d^2) = 1/d; conv = Erf(d/sqrt(2))
  DVE: u = conv * qb (qb = q_j broadcast across partitions);
       z = u * r with accum_out -> tloc[:, chunk]  (free-axis reduce)
One [128, CH] DMA per iteration returns t rows for this core.
"""
import sys

sys.path.insert(0, "/opt/trn_rl_repo")

import numpy as np
import ml_dtypes

import concourse.bass as bass
import concourse.tile as tile
from concourse import bacc, mybir
from concourse.bass_utils import run_bass_kernel_spmd

N = 4096
NCORES = 8
R = N // NCORES          # rows per core
CH = R // 128            # 128-row chunks per core
SIGMA = 1.0
TWOPI = 2.0 * np.pi
SELF_C = 1.0 / (SIGMA * TWOPI**1.5)
INV_SQRT2 = float(1.0 / np.sqrt(2.0))
BIG = float(2.0**40)

_nc_cache = None


def _build_nc(loop_n=None, unroll=1):
    nc = bacc.Bacc("TRN2", target_bir_lowering=False, debug=False,
                   num_devices=NCORES)
    f32 = mybir.dt.float32
    bf16 = mybir.dt.bfloat16
    E = mybir.ActivationFunctionType
    mult = mybir.AluOpType.mult
    add = mybir.AluOpType.add

    aug_d = nc.dram_tensor("aug", [5, R + N], f32, kind="ExternalInput").ap()
    qb_d = nc.dram_tensor("qb", [128, N], f32, kind="ExternalInput").ap()
    idn_d = nc.dram_tensor("idn", [128, 256], bf16, kind="ExternalInput").ap()
    t_d = nc.dram_tensor("t", [128, CH], f32, kind="ExternalOutput").ap()

    with tile.TileContext(nc) as tc:
        with (
            tc.tile_pool(name="sbin", bufs=1) as sbin,
            tc.tile_pool(name="psum", bufs=1, space="PSUM") as psum,
            tc.tile_pool(name="sbd", bufs=1) as sbd,
            tc.tile_pool(name="sbr", bufs=2) as sbr,
            tc.tile_pool(name="sbc", bufs=2) as sbc,
            tc.tile_pool(name="sbu", bufs=1) as sbu,
            tc.tile_pool(name="sbz", bufs=1) as sbz,
            tc.tile_pool(name="sbt", bufs=2) as sbt,
        ):
            aug = sbin.tile([5, R + N], f32, tag="aug")
            nc.sync.dma_start(aug[:], aug_d[:])
            qb = sbin.tile([128, N], f32, tag="qb")
            nc.sync.dma_start(qb[:], qb_d[:])
            idn = sbin.tile([128, 256], bf16, tag="idn")
            nc.sync.dma_start(idn[:], idn_d[:])

            def body():
                tloc = sbt.tile([128, CH], f32, tag="tloc")
                for ic in range(CH):
                    p = psum.tile([128, N], f32, tag="p")
                    for k4 in range(N // 512):
                        nc.tensor.matmul(
                            p[:, k4 * 512:(k4 + 1) * 512],
                            aug[:, ic * 128:(ic + 1) * 128],
                            aug[:, R + k4 * 512:R + (k4 + 1) * 512],
                            start=True, stop=not (k4 == 0))
                        if k4 == 0:
                            nc.tensor.matmul(
                                p[:, ic * 128:(ic + 1) * 128],
                                idn[:, 0:128], idn[:, 128:256],
                                start=False, stop=True)
                    d = sbd.tile([128, N], f32, tag="d")
                    nc.scalar.activation(d[:], p[:], E.Sqrt)
                    r = sbr.tile([128, N], f32, tag="r")
                    nc.scalar.activation(r[:], p[:], E.Abs_reciprocal_sqrt)
                    conv = sbc.tile([128, N], f32, tag="conv")
                    nc.scalar.activation(conv[:], d[:], E.Erf,
                                         scale=INV_SQRT2)
                    u = sbu.tile([128, N], f32, tag="u")
                    nc.vector.tensor_tensor(u[:], conv[:], qb[:], op=mult)
                    z = sbz.tile([128, N], f32, tag="z")
                    nc.vector.scalar_tensor_tensor(
                        out=z[:], in0=u[:], scalar=1.0, in1=r[:],
                        op0=mult, op1=mult,
                        accum_out=tloc[:, ic:ic + 1])
                nc.sync.dma_start(t_d[:], tloc[:])

            if loop_n is not None:
                with tc.For_i(0, loop_n, 1):
                    for _ in range(unroll):
                        body()
            else:
                body()
    nc.compile()
    return nc


def _prep_inputs(positions, q):
    pos = np.ascontiguousarray(np.asarray(positions, dtype=np.float32))
    qv = np.asarray(q, dtype=np.float32).reshape(-1)
    s = (pos * pos).sum(axis=1, dtype=np.float32)

    rhs_all = np.empty((5, N), np.float32)
    rhs_all[0:3] = pos.T
    rhs_all[3] = 1.0
    rhs_all[4] = s

    idn = np.concatenate([np.eye(128), np.eye(128) * BIG],
                         axis=1).astype(ml_dtypes.bfloat16)

    in_maps = []
    for c in range(NCORES):
        blk = slice(c * R, (c + 1) * R)
        aug = np.empty((5, R + N), np.float32)
        aug[0:3, 0:R] = -2.0 * pos[blk].T
        aug[3, 0:R] = s[blk]
        aug[4, 0:R] = 1.0
        aug[:, R:] = np.roll(rhs_all, -c * R, axis=1)
        qb = np.ascontiguousarray(
            np.broadcast_to(np.roll(qv, -c * R)[None, :], (128, N)),
            dtype=np.float32)
        in_maps.append({"aug": aug, "qb": qb, "idn": idn})
    return in_maps, qv


def kernel(positions, q):
    global _nc_cache
    if _nc_cache is None:
        _nc_cache = _build_nc()
    nc = _nc_cache

    in_maps, qv = _prep_inputs(positions, q)
    res = run_bass_kernel_spmd(nc, in_maps, core_ids=list(range(NCORES)))

    t = np.empty(N, np.float64)
    for c in range(NCORES):
        seg = res.results[c]["t"].astype(np.float64)  # [128, CH]
        t[c * R:(c + 1) * R] = seg.T.reshape(R)

    q64 = qv.astype(np.float64)
    field = t / TWOPI + 2.0 * SELF_C * q64
    pot = float((q64 * t).sum() / (2.0 * TWOPI) + SELF_C * (q64 * q64).sum())
    out = np.empty(N + 1, np.float32)
    out[0] = pot
    out[1:] = field.astype(np.float32)
    return out
